# revision 69
# baseline (speedup 1.0000x reference)
"""Trainium2 Bass kernel for causal MHA + RoPE (B=1, S=2048, E=2048, H=16, D=128).

Sharding: tensor-parallel over heads, 2 heads per core; Wq/Wk/Wv column-
sharded, Wo row-sharded, partials summed on host.

Per-core pipeline (cost-model-aware):
  - All large GEMMs run as fp8e4 DoubleRow matmuls (K=256/MM, 0.5 cyc/row)
    with hi+lo error compensation: for operands A=A_hi+A_lo, B=B_hi+B_lo the
    product is computed as A_hi*B_hi + A_lo*B_hi + A_hi*B_lo (3 passes, ~25%
    cheaper than one bf16 pass and more accurate). Weights are pre-scaled by
    powers of two on the host so fp8 values stay in the normal range; the
    inverse scales are folded into the RoPE tables / reciprocal constant /
    final host scaling.
  - Scores stay fp16 (K=128 contraction can't DoubleRow without a repack).
  - Causal: fully-masked key blocks skipped, diagonal blocks trimmed on the
    left (scores/exp/PV/denom all use the trimmed q-range); the remaining
    triangular wedge is a single [128,128] fp16 mask multiply.
  - Normalization: DVE reciprocal, fp16 ones-matmul partition broadcast.
  - The last strip's attention runs as two 256-wide q-halves so its
    out-projection tiles drain during the second half instead of fully
    trailing the kernel.
"""

import math
from contextlib import ExitStack

import numpy as np
import ml_dtypes

import concourse.bass as bass
import concourse.mybir as mybir
import concourse.tile as tile
from concourse.bass_utils import run_bass_kernel_spmd

F16 = mybir.dt.float16
F32 = mybir.dt.float32
F8 = mybir.dt.float8e4
AF = mybir.ActivationFunctionType
DRm = mybir.MatmulPerfMode.DoubleRow
E4 = ml_dtypes.float8_e4m3

S = 2048
E = 2048
D = 128          # head dim
NCORES = 8
HPC = 2          # heads per core
F2 = HPC * D     # 256 per-core qkv features
EC = E // 128    # 16 e-chunks of 128 (8 DoubleRow chunks of 256)
NQ = 512         # query tile width
NJ = S // NQ     # 4 query strips
NKB = S // 128   # 16 key blocks

SW = 256.0       # Wq/Wk host scale (undone via rope tables)
SV = 32.0        # Wv host scale (undone via bcast const)
SA = 16.0        # attn scale (applied via bcast const)
SO = 64.0        # Wo host scale (undone on host)
OUT_SCALE = SA * SO


def build_nc() -> bass.Bass:
    nc = bass.Bass()

    xh = nc.dram_tensor("xh", [128, EC, S], F8, kind="ExternalInput")
    xl = nc.dram_tensor("xl", [128, EC, S], F8, kind="ExternalInput")
    wqh = nc.dram_tensor("wqh", [128, EC * F2], F8, kind="ExternalInput")
    wql = nc.dram_tensor("wql", [128, EC * F2], F8, kind="ExternalInput")
    wkh = nc.dram_tensor("wkh", [128, EC * F2], F8, kind="ExternalInput")
    wkl = nc.dram_tensor("wkl", [128, EC * F2], F8, kind="ExternalInput")
    wvh = nc.dram_tensor("wvh", [128, EC * F2], F8, kind="ExternalInput")
    wvl = nc.dram_tensor("wvl", [128, EC * F2], F8, kind="ExternalInput")
    woh = nc.dram_tensor("woh", [128, HPC, E], F8, kind="ExternalInput")
    wol = nc.dram_tensor("wol", [128, HPC, E], F8, kind="ExternalInput")
    cosP = nc.dram_tensor("cosP", [D, S], F16, kind="ExternalInput")
    sinP = nc.dram_tensor("sinP", [D, S], F16, kind="ExternalInput")
    wedge = nc.dram_tensor("wedge", [128, 128], F16, kind="ExternalInput")
    ones8 = nc.dram_tensor("ones8", [128, 16], F16, kind="ExternalInput")
    ones16 = nc.dram_tensor("ones16", [1, 128], F16, kind="ExternalInput")
    biasm2 = nc.dram_tensor("biasm2", [128, 1], F32, kind="ExternalInput")
    out = nc.dram_tensor("out", [S, E], F16, kind="ExternalOutput")

    with tile.TileContext(nc) as tc:
        _emit(nc, tc, locals())
    _split_multi_waits(nc)
    return nc


def _split_multi_waits(nc):
    """Walrus codegen allows one sync-wait per TPB instruction. Split extras
    into wait-only EventSemaphore nops on the same engine."""
    for fn in nc.m.functions:
        for blk in fn.blocks:
            out_insts = []
            for inst in blk.instructions:
                si = inst.sync_info
                if si is not None and si.on_wait and len(si.on_wait) > 1:
                    waits = list(si.on_wait)
                    for k, w in enumerate(waits[:-1]):
                        ev = mybir.InstEventSemaphore(name=f"{inst.name}-ws{k}")
                        ev.engine = inst.engine
                        ev.sync_info = mybir.SyncInfo(on_wait=[w], on_update=[])
                        out_insts.append(ev)
                    inst.sync_info = mybir.SyncInfo(
                        on_wait=[waits[-1]], on_update=list(si.on_update or [])
                    )
                out_insts.append(inst)
            blk.instructions = out_insts


def _emit(nc, tc, io):
    with ExitStack() as ctx:
        consts = ctx.enter_context(tc.tile_pool(name="consts", bufs=1))
        state = ctx.enter_context(tc.tile_pool(name="state", bufs=1))
        tmps = ctx.enter_context(tc.tile_pool(name="tmps", bufs=3))
        psS = ctx.enter_context(tc.tile_pool(name="psS", bufs=3, space="PSUM"))
        psV = ctx.enter_context(tc.tile_pool(name="psV", bufs=2, space="PSUM"))
        psD = ctx.enter_context(tc.tile_pool(name="psD", bufs=1, space="PSUM"))
        psO = ctx.enter_context(tc.tile_pool(name="psO", bufs=2, space="PSUM"))

        # ---- SBUF tensors ----
        xh_sb = consts.tile([128, EC, S], F8)
        xl_sb = consts.tile([128, EC, S], F8)
        wq_sb = {0: consts.tile([128, EC, F2], F8, name="wq_h"),
                 1: consts.tile([128, EC, F2], F8, name="wq_l")}
        wk_sb = {0: consts.tile([128, EC, F2], F8, name="wk_h"),
                 1: consts.tile([128, EC, F2], F8, name="wk_l")}
        wv_sb = {0: consts.tile([128, EC, F2], F8, name="wv_h"),
                 1: consts.tile([128, EC, F2], F8, name="wv_l")}
        wo_sb = {0: consts.tile([128, HPC, E], F8, name="wo_h"),
                 1: consts.tile([128, HPC, E], F8, name="wo_l")}
        cos_sb = consts.tile([D, S], F16)
        sin_sb = consts.tile([D, S], F16)
        wedge_sb = consts.tile([128, 128], F16)
        ones8_sb = consts.tile([128, 16], F16)
        ones16_sb = consts.tile([1, 128], F16)
        bias_sb = consts.tile([128, 1], F32)

        QrT = state.tile([D, HPC, S], F16)
        KrT = state.tile([D, HPC, S], F16)
        V16 = state.tile([128, NKB, F2], F16)
        eP = state.tile([128, NKB, NQ], F16)
        attn_h = state.tile([D, HPC, S], F8)
        attn_l = state.tile([D, HPC, S], F8)
        ost_ring = state.tile([128, 4, 4, NQ], F16)
        ost_i = [0]

        dma = nc.sync.dma_start

        # one-time zero of the never-written left regions of diagonal eP
        # blocks (slot kb is diagonal in strip kb//4 with left trim 128*(kb%4))
        for kb in range(NKB):
            i = kb % 4
            if i >= 1:
                nc.gpsimd.memset(eP[:, kb, 0:128 * i], 0.0)

        # ---- constant/weight DMAs (order = need order) ----
        def dma_x(st, which=(0, 1)):
            sl = slice(st * NQ, (st + 1) * NQ)
            for w in which:
                sb, t = (xh_sb, io["xh"]) if w == 0 else (xl_sb, io["xl"])
                for cq in range(4):
                    dma(sb[:, 4 * cq:4 * cq + 4, sl], t[:, 4 * cq:4 * cq + 4, sl])

        dma(wq_sb[0][:, 0:8, :], io["wqh"][:, 0:8 * F2])
        dma(xh_sb[:, 0:8, 0:NQ], io["xh"][:, 0:8, 0:NQ])
        dma(wq_sb[1][:, 0:8, :], io["wql"][:, 0:8 * F2])
        dma(xl_sb[:, 0:8, 0:NQ], io["xl"][:, 0:8, 0:NQ])
        dma(wq_sb[0][:, 8:16, :], io["wqh"][:, 8 * F2:16 * F2])
        dma(wq_sb[1][:, 8:16, :], io["wql"][:, 8 * F2:16 * F2])
        dma(xh_sb[:, 8:16, 0:NQ], io["xh"][:, 8:16, 0:NQ])
        dma(xl_sb[:, 8:16, 0:NQ], io["xl"][:, 8:16, 0:NQ])
        # strip-0 projections run Q-f0, Q-f1 (wq+x only) before K, so wk may
        # stream during Q-f1; RoPE needs cos/sin only ~10us in.
        dma(wk_sb[0][:, :, :], io["wkh"][:, :])
        dma(wk_sb[1][:, :, :], io["wkl"][:, :])
        dma(cos_sb[:, 0:NQ], io["cosP"][:, 0:NQ])
        dma(sin_sb[:, 0:NQ], io["sinP"][:, 0:NQ])
        dma(bias_sb[:, :], io["biasm2"][:, :])
        dma(ones8_sb[:, :], io["ones8"][:, :])
        dma(ones16_sb[:, :], io["ones16"][:, :])
        dma(wedge_sb[:, :], io["wedge"][:, :])
        dma(wv_sb[0][:, :, :], io["wvh"][:, :])
        dma(wv_sb[1][:, :, :], io["wvl"][:, :])
        dma(cos_sb[:, NQ:2 * NQ], io["cosP"][:, NQ:2 * NQ])
        dma(sin_sb[:, NQ:2 * NQ], io["sinP"][:, NQ:2 * NQ])
        dma_x(1)
        dma(cos_sb[:, 2 * NQ:S], io["cosP"][:, 2 * NQ:S])
        dma(sin_sb[:, 2 * NQ:S], io["sinP"][:, 2 * NQ:S])
        dma(wo_sb[0][:, :, :], io["woh"][:, :, :])
        dma(wo_sb[1][:, :, :], io["wol"][:, :, :])
        dma_x(2)
        dma_x(3)

        # ---- projection helpers ----
        def dr_terms(ps, lhs_pair, rhs_pair, drop_lo_ci=()):
            """3-term hi/lo-compensated DoubleRow accumulation over 8 K-chunks.
            lhs_pair/rhs_pair: (hi_fn, lo_fn) returning [128,2,*] APs per chunk.
            drop_lo_ci: chunk-pairs whose lhs-lo term is skipped (validated
            error trade: ~1.3e-2 max rel for Q/K with pairs 0-3 dropped)."""
            lh, ll = lhs_pair
            rh, rl = rhs_pair
            order = []
            for ci in range(EC // 2):
                order.append((ci, (lh, rh)))
                if ci not in drop_lo_ci:
                    order.append((ci, (ll, rh)))
                order.append((ci, (lh, rl)))
            n = len(order)
            for i, (ci, (lf, rf)) in enumerate(order):
                nc.tensor.matmul(
                    ps, lhsT=lf(ci), rhs=rf(ci),
                    start=(i == 0), stop=(i == n - 1), perf_mode=DRm,
                )

        def qk_group(w_sb2, dstT, f, st):
            sl = slice(st * NQ, (st + 1) * NQ)
            fs = slice(f * 128, (f + 1) * 128)
            ps = psS.tile([128, NQ], F32, tag="S", name="ps_proj")
            dr_terms(
                ps,
                (lambda ci, w=w_sb2[0]: w[:, 2 * ci:2 * ci + 2, fs],
                 lambda ci, w=w_sb2[1]: w[:, 2 * ci:2 * ci + 2, fs]),
                (lambda ci: xh_sb[:, 2 * ci:2 * ci + 2, sl],
                 lambda ci: xl_sb[:, 2 * ci:2 * ci + 2, sl]),
                drop_lo_ci=(),
            )
            q16 = tmps.tile([128, NQ], F16, tag="q16", name="q16")
            nc.scalar.copy(q16, ps)
            t1 = tmps.tile([128, NQ], F16, tag="ropeA", name="t1")
            t2 = tmps.tile([128, NQ], F16, tag="ropeB", name="t2")
            nc.vector.tensor_mul(t1[0:64, :], ps[64:128, :], sin_sb[0:64, sl])
            nc.vector.tensor_mul(t1[64:128, :], ps[0:64, :], sin_sb[64:128, sl])
            nc.gpsimd.tensor_mul(t2, q16, cos_sb[:, sl])
            nc.vector.tensor_add(dstT[:, f, sl], t1, t2)

        def v_group(sc):
            psv = psV.tile([128, F2], F32, tag="V", name="ps_v")
            scs = slice(sc * 128, (sc + 1) * 128)
            dr_terms(
                psv,
                (lambda ci: xh_sb[:, 2 * ci:2 * ci + 2, scs],
                 lambda ci: xl_sb[:, 2 * ci:2 * ci + 2, scs]),
                (lambda ci: wv_sb[0][:, 2 * ci:2 * ci + 2, :],
                 lambda ci: wv_sb[1][:, 2 * ci:2 * ci + 2, :]),
            )
            if sc % 2 == 0:
                nc.scalar.copy(V16[:, sc, :], psv)
            else:
                nc.vector.tensor_copy(V16[:, sc, :], psv)

        # ---- outproj ----
        pending = []

        def emit_outproj(sc, ec, pool=None, tag="O", flush=False):
            pool = pool or psO
            pso = pool.tile([128, NQ], F32, tag=tag, name="pso")
            scs = slice(sc * 128, (sc + 1) * 128)
            ecs = slice(ec * NQ, (ec + 1) * NQ)
            for i, (a, w) in enumerate(
                ((attn_h, wo_sb[0]), (attn_l, wo_sb[0]), (attn_h, wo_sb[1]))
            ):
                nc.tensor.matmul(
                    pso, lhsT=a[:, :, scs], rhs=w[:, :, ecs],
                    start=(i == 0), stop=(i == 2), perf_mode=DRm,
                )
            oi = ost_i[0]
            ost_i[0] += 1
            row = (oi // 4) % 4
            ost = ost_ring[:, row, ec, :]
            if (oi % 2 == 0) if flush else False:
                nc.scalar.copy(ost, pso)
            else:
                nc.vector.tensor_copy(ost, pso)
            if flush:
                if ec % 2 == 1:
                    esl = slice((ec - 1) * NQ, (ec + 1) * NQ)
                    dma(io["out"][scs, esl], ost_ring[:, row, ec - 1:ec + 1, :])
            elif ec == 3:
                dma(io["out"][scs, :], ost_ring[:, row, :, :])

        def drain_pending(n=1):
            for _ in range(min(n, len(pending))):
                emit_outproj(*pending.pop(0))

        # ---- attention pass over q sub-window [qa, qb) of strip j ----
        def scores_blk(j, h, kb, qa, qb):
            base = j * NQ
            v = kb * 128 - base    # strip-local col where block becomes visible
            q0 = max(qa, v)
            n = qb - q0
            ps_s = psS.tile([128, NQ], F32, tag="S", name="ps_s")
            nc.tensor.matmul(
                ps_s[:, 0:n],
                lhsT=KrT[:, h, kb * 128:(kb + 1) * 128],
                rhs=QrT[:, h, base + q0:base + qb],
                start=True, stop=True,
            )
            nc.scalar.activation(
                eP[:, kb, q0:qb], ps_s[:, 0:n], AF.Exp, bias=bias_sb
            )
            if v >= qa:
                eng = nc.vector if j == NJ - 1 else nc.gpsimd
                eng.tensor_mul(
                    eP[:, kb, v:v + 128], eP[:, kb, v:v + 128], wedge_sb
                )

        def attn_pass(j, h, qa, qb, drain_per_pair, drain_budget=None, pre=0):
            budget = [len(pending) if drain_budget is None else drain_budget]
            base = j * NQ
            nblk_w = (base + qb) // 128
            npair = nblk_w // 2
            ps_d = psD.tile([128, NQ], F32, tag="D", name="ps_d")
            ps_o = psV.tile([128, NQ], F32, tag="V", name="ps_o")

            def scores(kb):
                if kb < pre:
                    return
                scores_blk(j, h, kb, qa, qb)

            def accum(pr):
                for kb in (2 * pr, 2 * pr + 1):
                    v = kb * 128 - base
                    q0 = max(qa, v)
                    rhs = eP[:, kb, q0:qb]
                    nc.tensor.matmul(
                        ps_d[0:1, q0:qb], lhsT=ones8_sb[:, 0:1], rhs=rhs,
                        start=(kb == 0), stop=(kb == nblk_w - 1),
                    )
                    nc.tensor.matmul(
                        ps_o[:, q0:qb],
                        lhsT=V16[:, kb, h * 128:(h + 1) * 128],
                        rhs=rhs,
                        start=(kb == 0), stop=(kb == nblk_w - 1),
                    )

            # software pipeline: scores four blocks ahead of accumulation
            scores(0)
            scores(1)
            if nblk_w > 2:
                scores(2)
            if nblk_w > 3:
                scores(3)
            for pr in range(npair):
                if 2 * pr + 4 < nblk_w:
                    scores(2 * pr + 4)
                if 2 * pr + 5 < nblk_w:
                    scores(2 * pr + 5)
                accum(pr)
                nd = min(drain_per_pair, budget[0])
                drain_pending(nd)
                budget[0] -= nd

            rec16 = tmps.tile([1, NQ], F16, tag="rec16", name="rec16")
            with nc.allow_low_precision(reason="fp16 reciprocal; 0.05% rel"):
                nc.vector.reciprocal(rec16[:, qa:qb], ps_d[0:1, qa:qb])
            ps_b = psD.tile([128, NQ], F32, tag="D", name="ps_b")
            nc.tensor.matmul(ps_b[:, qa:qb], lhsT=ones16_sb,
                             rhs=rec16[:, qa:qb], start=True, stop=True)
            bc = tmps.tile([128, NQ], F16, tag="bc", name="bc")
            at = tmps.tile([128, NQ], F16, tag="at", name="attn_tmp")
            if j == NJ - 1:
                nc.scalar.copy(bc[:, qa:qb], ps_b[:, qa:qb])
            else:
                nc.vector.tensor_copy(bc[:, qa:qb], ps_b[:, qa:qb])
            nc.vector.tensor_mul(at[:, qa:qb], ps_o[:, qa:qb], bc[:, qa:qb])
            g = slice(base + qa, base + qb)
            nc.scalar.copy(attn_h[:, h, g], at[:, qa:qb])
            nc.vector.tensor_sub(attn_l[:, h, g], at[:, qa:qb], attn_h[:, h, g])

        # ---- main pipeline: per s-strip, projections then attention ----
        for st in range(NJ):
            # projections for this strip
            if st == 0:
                qk_group(wq_sb, QrT, 0, st)
                qk_group(wq_sb, QrT, 1, st)
                qk_group(wk_sb, KrT, 0, st)
                qk_group(wk_sb, KrT, 1, st)
                for sc in range(4):
                    v_group(sc)
            else:
                qk_group(wq_sb, QrT, 0, st)
                qk_group(wk_sb, KrT, 0, st)
                v_group(4 * st + 0)
                v_group(4 * st + 1)
                qk_group(wq_sb, QrT, 1, st)
                qk_group(wk_sb, KrT, 1, st)
                v_group(4 * st + 2)
                v_group(4 * st + 3)

            j = st
            if j < NJ - 1:
                attn_pass(j, 0, 0, NQ, drain_per_pair=2 if j < 2 else 1)
                attn_pass(j, 1, 0, NQ, drain_per_pair=2 if j < 2 else 1)
                for sc in range(4 * j, 4 * j + 4):
                    for ec in range(4):
                        pending.append((sc, ec))
            else:
                attn_pass(j, 0, 0, NQ, drain_per_pair=1)
                attn_pass(j, 1, 0, NQ, drain_per_pair=1)
                for sc in range(4 * j, 4 * j + 4):
                    for ec in range(4):
                        pending.append((sc, ec))
                pools = [(psO, "O"), (psS, "S"), (psV, "V")]
                k = 0
                while pending:
                    pool, tag = pools[k % 3]
                    emit_outproj(*pending.pop(0), pool=pool, tag=tag, flush=True)
                    k += 1


_NC_CACHE = None


def _get_nc():
    global _NC_CACHE
    if _NC_CACHE is None:
        _NC_CACHE = build_nc()
    return _NC_CACHE


def _split8(a):
    hi = a.astype(E4)
    lo = (a - hi.astype(np.float32)).astype(E4)
    return hi, lo


def _pack_e(a):
    """[E, N] -> [128, EC, N] with e = c*128 + p."""
    n = a.shape[1]
    return np.ascontiguousarray(a.reshape(EC, 128, n).transpose(1, 0, 2))


def _prep_inputs(x, rotary_cos, rotary_sin, Wq, Wk, Wv, Wo):
    x = np.asarray(x, dtype=np.float32)[0]          # [S, E]
    cos = np.asarray(rotary_cos, dtype=np.float32)[0]  # [S, D]
    sin = np.asarray(rotary_sin, dtype=np.float32)[0]
    Wq = np.asarray(Wq, dtype=np.float32)
    Wk = np.asarray(Wk, dtype=np.float32)
    Wv = np.asarray(Wv, dtype=np.float32)
    Wo = np.asarray(Wo, dtype=np.float32)

    xT = np.ascontiguousarray(x.T)                   # [E, S]
    xh, xl = _split8(xT)
    xh = _pack_e(xh.astype(np.float32)).astype(E4)
    xl = _pack_e(xl.astype(np.float32)).astype(E4)

    cq = 1.0 / (SW * math.sqrt(math.sqrt(D)))
    cosP = np.ascontiguousarray(cos.T * cq).astype(np.float16)
    sinT = sin.T * cq
    sinP = np.concatenate([-sinT[:64], sinT[64:]], axis=0)
    sinP = np.ascontiguousarray(sinP).astype(np.float16)

    kk = np.arange(128)[:, None]
    qq = np.arange(128)[None, :]
    wedge = (kk <= qq).astype(np.float16)
    ones8 = np.ones((128, 16), dtype=np.float16)
    ones16 = np.full((1, 128), SA / SV, dtype=np.float16)
    biasm2 = np.full((128, 1), -2.0, dtype=np.float32)

    in_maps = []
    for c in range(NCORES):
        fs = slice(F2 * c, F2 * (c + 1))
        wq_h, wq_l = _split8(Wq[fs, :].T * SW)       # [E, F2]
        wk_h, wk_l = _split8(Wk[fs, :].T * SW)
        wv_h, wv_l = _split8(Wv[fs, :].T * SV)
        wo_s = Wo[:, fs].T * SO                       # [F2, E]
        wo_h, wo_l = _split8(wo_s)
        pk = lambda a: _pack_e(a.astype(np.float32)).astype(E4).reshape(128, EC * F2)
        pko = lambda a: np.ascontiguousarray(
            a.astype(np.float32).reshape(HPC, 128, E).transpose(1, 0, 2)
        ).astype(E4)
        in_maps.append({
            "xh": xh, "xl": xl,
            "wqh": pk(wq_h), "wql": pk(wq_l),
            "wkh": pk(wk_h), "wkl": pk(wk_l),
            "wvh": pk(wv_h), "wvl": pk(wv_l),
            "woh": pko(wo_h), "wol": pko(wo_l),
            "cosP": cosP, "sinP": sinP, "wedge": wedge,
            "ones8": ones8, "ones16": ones16, "biasm2": biasm2,
        })
    return in_maps


def kernel(x, rotary_cos, rotary_sin, Wq, Wk, Wv, Wo, **run_kwargs):
    nc = _get_nc()
    in_maps = _prep_inputs(x, rotary_cos, rotary_sin, Wq, Wk, Wv, Wo)
    res = run_bass_kernel_spmd(nc, in_maps, core_ids=list(range(NCORES)), **run_kwargs)
    acc = np.zeros((S, E), dtype=np.float64)
    for r in res.results:
        acc += r["out"].astype(np.float64)
    full = (acc / OUT_SCALE).astype(np.float32).reshape(1, S, E)
    if run_kwargs:
        return full, res
    return full


# revision 71
# speedup vs baseline: 1.0147x; 1.0147x over previous
"""Trainium2 Bass kernel for causal MHA + RoPE (B=1, S=2048, E=2048, H=16, D=128).

Sharding: tensor-parallel over heads, 2 heads per core; Wq/Wk/Wv column-
sharded, Wo row-sharded, partials summed on host.

Per-core pipeline (cost-model-aware):
  - All large GEMMs run as fp8e4 DoubleRow matmuls (K=256/MM, 0.5 cyc/row)
    with hi+lo error compensation: for operands A=A_hi+A_lo, B=B_hi+B_lo the
    product is computed as A_hi*B_hi + A_lo*B_hi + A_hi*B_lo (3 passes, ~25%
    cheaper than one bf16 pass and more accurate). Weights are pre-scaled by
    powers of two on the host so fp8 values stay in the normal range; the
    inverse scales are folded into the RoPE tables / reciprocal constant /
    final host scaling.
  - Scores stay fp16 (K=128 contraction can't DoubleRow without a repack).
  - Causal: fully-masked key blocks skipped, diagonal blocks trimmed on the
    left (scores/exp/PV/denom all use the trimmed q-range); the remaining
    triangular wedge is a single [128,128] fp16 mask multiply.
  - Normalization: DVE reciprocal, fp16 ones-matmul partition broadcast.
  - The last strip's attention runs as two 256-wide q-halves so its
    out-projection tiles drain during the second half instead of fully
    trailing the kernel.
"""

import math
from contextlib import ExitStack

import numpy as np
import ml_dtypes

import concourse.bass as bass
import concourse.mybir as mybir
import concourse.tile as tile
from concourse.bass_utils import run_bass_kernel_spmd

F16 = mybir.dt.float16
F32 = mybir.dt.float32
F8 = mybir.dt.float8e4
AF = mybir.ActivationFunctionType
DRm = mybir.MatmulPerfMode.DoubleRow
E4 = ml_dtypes.float8_e4m3

S = 2048
E = 2048
D = 128          # head dim
NCORES = 8
HPC = 2          # heads per core
F2 = HPC * D     # 256 per-core qkv features
EC = E // 128    # 16 e-chunks of 128 (8 DoubleRow chunks of 256)
NQ = 512         # query tile width
NJ = S // NQ     # 4 query strips
NKB = S // 128   # 16 key blocks

SW = 256.0       # Wq/Wk host scale (undone via rope tables)
SV = 32.0        # Wv host scale (undone via bcast const)
SA = 16.0        # attn scale (applied via bcast const)
SO = 64.0        # Wo host scale (undone on host)
OUT_SCALE = SA * SO


def build_nc() -> bass.Bass:
    nc = bass.Bass()

    xh = nc.dram_tensor("xh", [128, EC, S], F8, kind="ExternalInput")
    xl = nc.dram_tensor("xl", [128, EC, S], F8, kind="ExternalInput")
    wqh = nc.dram_tensor("wqh", [128, EC * F2], F8, kind="ExternalInput")
    wql = nc.dram_tensor("wql", [128, EC * F2], F8, kind="ExternalInput")
    wkh = nc.dram_tensor("wkh", [128, EC * F2], F8, kind="ExternalInput")
    wkl = nc.dram_tensor("wkl", [128, EC * F2], F8, kind="ExternalInput")
    wvh = nc.dram_tensor("wvh", [128, EC * F2], F8, kind="ExternalInput")
    wvl = nc.dram_tensor("wvl", [128, EC * F2], F8, kind="ExternalInput")
    woh = nc.dram_tensor("woh", [128, HPC, E], F8, kind="ExternalInput")
    wol = nc.dram_tensor("wol", [128, HPC, E], F8, kind="ExternalInput")
    cosP = nc.dram_tensor("cosP", [D, S], F16, kind="ExternalInput")
    sinP = nc.dram_tensor("sinP", [D, S], F16, kind="ExternalInput")
    wedge = nc.dram_tensor("wedge", [128, 128], F16, kind="ExternalInput")
    ones8 = nc.dram_tensor("ones8", [128, 16], F16, kind="ExternalInput")
    ones16 = nc.dram_tensor("ones16", [1, 128], F16, kind="ExternalInput")
    biasm2 = nc.dram_tensor("biasm2", [128, 1], F32, kind="ExternalInput")
    out = nc.dram_tensor("out", [S, E], F16, kind="ExternalOutput")

    with tile.TileContext(nc) as tc:
        _emit(nc, tc, locals())
    _split_multi_waits(nc)
    return nc


def _split_multi_waits(nc):
    """Walrus codegen allows one sync-wait per TPB instruction. Split extras
    into wait-only EventSemaphore nops on the same engine."""
    for fn in nc.m.functions:
        for blk in fn.blocks:
            out_insts = []
            for inst in blk.instructions:
                si = inst.sync_info
                if si is not None and si.on_wait and len(si.on_wait) > 1:
                    waits = list(si.on_wait)
                    for k, w in enumerate(waits[:-1]):
                        ev = mybir.InstEventSemaphore(name=f"{inst.name}-ws{k}")
                        ev.engine = inst.engine
                        ev.sync_info = mybir.SyncInfo(on_wait=[w], on_update=[])
                        out_insts.append(ev)
                    inst.sync_info = mybir.SyncInfo(
                        on_wait=[waits[-1]], on_update=list(si.on_update or [])
                    )
                out_insts.append(inst)
            blk.instructions = out_insts


def _emit(nc, tc, io):
    with ExitStack() as ctx:
        consts = ctx.enter_context(tc.tile_pool(name="consts", bufs=1))
        state = ctx.enter_context(tc.tile_pool(name="state", bufs=1))
        tmps = ctx.enter_context(tc.tile_pool(name="tmps", bufs=3))
        psS = ctx.enter_context(tc.tile_pool(name="psS", bufs=3, space="PSUM"))
        psV = ctx.enter_context(tc.tile_pool(name="psV", bufs=2, space="PSUM"))
        psD = ctx.enter_context(tc.tile_pool(name="psD", bufs=1, space="PSUM"))
        psO = ctx.enter_context(tc.tile_pool(name="psO", bufs=2, space="PSUM"))

        # ---- SBUF tensors ----
        xh_sb = consts.tile([128, EC, S], F8)
        xl_sb = consts.tile([128, EC, S], F8)
        wq_sb = {0: consts.tile([128, EC, F2], F8, name="wq_h"),
                 1: consts.tile([128, EC, F2], F8, name="wq_l")}
        wk_sb = {0: consts.tile([128, EC, F2], F8, name="wk_h"),
                 1: consts.tile([128, EC, F2], F8, name="wk_l")}
        wv_sb = {0: consts.tile([128, EC, F2], F8, name="wv_h"),
                 1: consts.tile([128, EC, F2], F8, name="wv_l")}
        wo_sb = {0: consts.tile([128, HPC, E], F8, name="wo_h"),
                 1: consts.tile([128, HPC, E], F8, name="wo_l")}
        cos_sb = consts.tile([D, S], F16)
        sin_sb = consts.tile([D, S], F16)
        wedge_sb = consts.tile([128, 128], F16)
        ones8_sb = consts.tile([128, 16], F16)
        ones16_sb = consts.tile([1, 128], F16)
        bias_sb = consts.tile([128, 1], F32)

        QrT = state.tile([D, HPC, S], F16)
        KrT = state.tile([D, HPC, S], F16)
        V16 = state.tile([128, NKB, F2], F16)
        eP = state.tile([128, NKB, NQ], F16)
        attn_h = state.tile([D, HPC, S], F8)
        attn_l = state.tile([D, HPC, S], F8)
        ost_ring = state.tile([128, 4, 4, NQ], F16)
        ost_i = [0]

        dma = nc.sync.dma_start

        # one-time zero of the never-written left regions of diagonal eP
        # blocks (slot kb is diagonal in strip kb//4 with left trim 128*(kb%4))
        for kb in range(NKB):
            i = kb % 4
            if i >= 1:
                nc.gpsimd.memset(eP[:, kb, 0:128 * i], 0.0)

        # ---- constant/weight DMAs (order = need order) ----
        def dma_x(st, which=(0, 1)):
            sl = slice(st * NQ, (st + 1) * NQ)
            for w in which:
                sb, t = (xh_sb, io["xh"]) if w == 0 else (xl_sb, io["xl"])
                for cq in range(4):
                    dma(sb[:, 4 * cq:4 * cq + 4, sl], t[:, 4 * cq:4 * cq + 4, sl])

        dma(wq_sb[0][:, 0:8, :], io["wqh"][:, 0:8 * F2])
        dma(xh_sb[:, 0:8, 0:NQ], io["xh"][:, 0:8, 0:NQ])
        dma(wq_sb[1][:, 0:8, :], io["wql"][:, 0:8 * F2])
        dma(xl_sb[:, 0:8, 0:NQ], io["xl"][:, 0:8, 0:NQ])
        dma(wq_sb[0][:, 8:16, :], io["wqh"][:, 8 * F2:16 * F2])
        dma(wq_sb[1][:, 8:16, :], io["wql"][:, 8 * F2:16 * F2])
        dma(xh_sb[:, 8:16, 0:NQ], io["xh"][:, 8:16, 0:NQ])
        dma(xl_sb[:, 8:16, 0:NQ], io["xl"][:, 8:16, 0:NQ])
        # strip-0 projections run Q-f0, Q-f1 (wq+x only) before K, so wk may
        # stream during Q-f1; RoPE needs cos/sin only ~10us in.
        dma(wk_sb[0][:, :, :], io["wkh"][:, :])
        dma(wk_sb[1][:, :, :], io["wkl"][:, :])
        dma(cos_sb[:, 0:NQ], io["cosP"][:, 0:NQ])
        dma(sin_sb[:, 0:NQ], io["sinP"][:, 0:NQ])
        dma(bias_sb[:, :], io["biasm2"][:, :])
        dma(ones8_sb[:, :], io["ones8"][:, :])
        dma(ones16_sb[:, :], io["ones16"][:, :])
        dma(wedge_sb[:, :], io["wedge"][:, :])
        dma(wv_sb[0][:, :, :], io["wvh"][:, :])
        dma(wv_sb[1][:, :, :], io["wvl"][:, :])
        dma(cos_sb[:, NQ:2 * NQ], io["cosP"][:, NQ:2 * NQ])
        dma(sin_sb[:, NQ:2 * NQ], io["sinP"][:, NQ:2 * NQ])
        dma_x(1)
        dma(cos_sb[:, 2 * NQ:S], io["cosP"][:, 2 * NQ:S])
        dma(sin_sb[:, 2 * NQ:S], io["sinP"][:, 2 * NQ:S])
        dma(wo_sb[0][:, :, :], io["woh"][:, :, :])
        dma(wo_sb[1][:, :, :], io["wol"][:, :, :])
        dma_x(2)
        dma_x(3)

        # ---- projection helpers ----
        def dr_terms(ps, lhs_pair, rhs_pair, drop_lo_ci=()):
            """3-term hi/lo-compensated DoubleRow accumulation over 8 K-chunks.
            lhs_pair/rhs_pair: (hi_fn, lo_fn) returning [128,2,*] APs per chunk.
            drop_lo_ci: chunk-pairs whose lhs-lo term is skipped (validated
            error trade: ~1.3e-2 max rel for Q/K with pairs 0-3 dropped)."""
            lh, ll = lhs_pair
            rh, rl = rhs_pair
            order = []
            for ci in range(EC // 2):
                order.append((ci, (lh, rh)))
                if ci not in drop_lo_ci:
                    order.append((ci, (ll, rh)))
                order.append((ci, (lh, rl)))
            n = len(order)
            for i, (ci, (lf, rf)) in enumerate(order):
                nc.tensor.matmul(
                    ps, lhsT=lf(ci), rhs=rf(ci),
                    start=(i == 0), stop=(i == n - 1), perf_mode=DRm,
                )

        def qk_group(w_sb2, dstT, f, st):
            sl = slice(st * NQ, (st + 1) * NQ)
            fs = slice(f * 128, (f + 1) * 128)
            ps = psS.tile([128, NQ], F32, tag="S", name="ps_proj")
            dr_terms(
                ps,
                (lambda ci, w=w_sb2[0]: w[:, 2 * ci:2 * ci + 2, fs],
                 lambda ci, w=w_sb2[1]: w[:, 2 * ci:2 * ci + 2, fs]),
                (lambda ci: xh_sb[:, 2 * ci:2 * ci + 2, sl],
                 lambda ci: xl_sb[:, 2 * ci:2 * ci + 2, sl]),
                drop_lo_ci=(),
            )
            q16 = tmps.tile([128, NQ], F16, tag="q16", name="q16")
            nc.scalar.copy(q16, ps)
            t1 = tmps.tile([128, NQ], F16, tag="ropeA", name="t1")
            t2 = tmps.tile([128, NQ], F16, tag="ropeB", name="t2")
            nc.vector.tensor_mul(t1[0:64, :], ps[64:128, :], sin_sb[0:64, sl])
            nc.vector.tensor_mul(t1[64:128, :], ps[0:64, :], sin_sb[64:128, sl])
            nc.gpsimd.tensor_mul(t2, q16, cos_sb[:, sl])
            nc.vector.tensor_add(dstT[:, f, sl], t1, t2)

        def v_group(sc):
            psv = psV.tile([128, F2], F32, tag="V", name="ps_v")
            scs = slice(sc * 128, (sc + 1) * 128)
            dr_terms(
                psv,
                (lambda ci: xh_sb[:, 2 * ci:2 * ci + 2, scs],
                 lambda ci: xl_sb[:, 2 * ci:2 * ci + 2, scs]),
                (lambda ci: wv_sb[0][:, 2 * ci:2 * ci + 2, :],
                 lambda ci: wv_sb[1][:, 2 * ci:2 * ci + 2, :]),
            )
            if sc % 2 == 0:
                nc.scalar.copy(V16[:, sc, :], psv)
            else:
                nc.vector.tensor_copy(V16[:, sc, :], psv)

        # ---- outproj ----
        pending = []

        def emit_outproj(sc, ec, pool=None, tag="O", flush=False):
            pool = pool or psO
            pso = pool.tile([128, NQ], F32, tag=tag, name="pso")
            scs = slice(sc * 128, (sc + 1) * 128)
            ecs = slice(ec * NQ, (ec + 1) * NQ)
            for i, (a, w) in enumerate(
                ((attn_h, wo_sb[0]), (attn_l, wo_sb[0]), (attn_h, wo_sb[1]))
            ):
                nc.tensor.matmul(
                    pso, lhsT=a[:, :, scs], rhs=w[:, :, ecs],
                    start=(i == 0), stop=(i == 2), perf_mode=DRm,
                )
            oi = ost_i[0]
            ost_i[0] += 1
            row = (oi // 4) % 4
            ost = ost_ring[:, row, ec, :]
            if (oi % 2 == 0) if flush else False:
                nc.scalar.copy(ost, pso)
            else:
                nc.vector.tensor_copy(ost, pso)
            if flush:
                if ec % 2 == 1:
                    esl = slice((ec - 1) * NQ, (ec + 1) * NQ)
                    dma(io["out"][scs, esl], ost_ring[:, row, ec - 1:ec + 1, :])
            elif ec == 3:
                dma(io["out"][scs, :], ost_ring[:, row, :, :])

        def drain_pending(n=1):
            for _ in range(min(n, len(pending))):
                emit_outproj(*pending.pop(0))

        # ---- attention pass over q sub-window [qa, qb) of strip j ----
        def scores_blk(j, h, kb, qa, qb):
            base = j * NQ
            v = kb * 128 - base    # strip-local col where block becomes visible
            q0 = max(qa, v)
            n = qb - q0
            ps_s = psS.tile([128, NQ], F32, tag="S", name="ps_s")
            nc.tensor.matmul(
                ps_s[:, 0:n],
                lhsT=KrT[:, h, kb * 128:(kb + 1) * 128],
                rhs=QrT[:, h, base + q0:base + qb],
                start=True, stop=True,
            )
            nc.scalar.activation(
                eP[:, kb, q0:qb], ps_s[:, 0:n], AF.Exp, bias=bias_sb
            )
            if v >= qa:
                eng = nc.vector if j == NJ - 1 else nc.gpsimd
                eng.tensor_mul(
                    eP[:, kb, v:v + 128], eP[:, kb, v:v + 128], wedge_sb
                )

        def attn_pass(j, h, qa, qb, drain_per_pair, drain_budget=None, pre=0):
            budget = [len(pending) if drain_budget is None else drain_budget]
            base = j * NQ
            nblk_w = (base + qb) // 128
            npair = nblk_w // 2
            ps_d = psD.tile([128, NQ], F32, tag="D", name="ps_d")
            ps_o = psV.tile([128, NQ], F32, tag="V", name="ps_o")

            def scores(kb):
                if kb < pre:
                    return
                scores_blk(j, h, kb, qa, qb)

            def accum(pr):
                for kb in (2 * pr, 2 * pr + 1):
                    v = kb * 128 - base
                    q0 = max(qa, v)
                    rhs = eP[:, kb, q0:qb]
                    nc.tensor.matmul(
                        ps_d[0:1, q0:qb], lhsT=ones8_sb[:, 0:1], rhs=rhs,
                        start=(kb == 0), stop=(kb == nblk_w - 1),
                    )
                    nc.tensor.matmul(
                        ps_o[:, q0:qb],
                        lhsT=V16[:, kb, h * 128:(h + 1) * 128],
                        rhs=rhs,
                        start=(kb == 0), stop=(kb == nblk_w - 1),
                    )

            # software pipeline: scores three blocks ahead of accumulation
            scores(0)
            scores(1)
            if nblk_w > 2:
                scores(2)
            for pr in range(npair):
                if 2 * pr + 3 < nblk_w:
                    scores(2 * pr + 3)
                if 2 * pr + 4 < nblk_w:
                    scores(2 * pr + 4)
                accum(pr)
                nd = min(drain_per_pair, budget[0])
                drain_pending(nd)
                budget[0] -= nd

            rec16 = tmps.tile([1, NQ], F16, tag="rec16", name="rec16")
            with nc.allow_low_precision(reason="fp16 reciprocal; 0.05% rel"):
                nc.vector.reciprocal(rec16[:, qa:qb], ps_d[0:1, qa:qb])
            ps_b = psD.tile([128, NQ], F32, tag="D", name="ps_b")
            nc.tensor.matmul(ps_b[:, qa:qb], lhsT=ones16_sb,
                             rhs=rec16[:, qa:qb], start=True, stop=True)
            bc = tmps.tile([128, NQ], F16, tag="bc", name="bc")
            at = tmps.tile([128, NQ], F16, tag="at", name="attn_tmp")
            if j == NJ - 1:
                nc.scalar.copy(bc[:, qa:qb], ps_b[:, qa:qb])
            else:
                nc.vector.tensor_copy(bc[:, qa:qb], ps_b[:, qa:qb])
            nc.vector.tensor_mul(at[:, qa:qb], ps_o[:, qa:qb], bc[:, qa:qb])
            g = slice(base + qa, base + qb)
            nc.scalar.copy(attn_h[:, h, g], at[:, qa:qb])
            nc.vector.tensor_sub(attn_l[:, h, g], at[:, qa:qb], attn_h[:, h, g])

        # ---- main pipeline: per s-strip, projections then attention ----
        for st in range(NJ):
            # projections for this strip
            if st == 0:
                qk_group(wq_sb, QrT, 0, st)
                qk_group(wq_sb, QrT, 1, st)
                qk_group(wk_sb, KrT, 0, st)
                qk_group(wk_sb, KrT, 1, st)
                for sc in range(4):
                    v_group(sc)
            else:
                qk_group(wq_sb, QrT, 0, st)
                qk_group(wk_sb, KrT, 0, st)
                v_group(4 * st + 0)
                v_group(4 * st + 1)
                qk_group(wq_sb, QrT, 1, st)
                qk_group(wk_sb, KrT, 1, st)
                v_group(4 * st + 2)
                v_group(4 * st + 3)

            j = st
            if j < NJ - 1:
                attn_pass(j, 0, 0, NQ, drain_per_pair=1)
                attn_pass(j, 1, 0, NQ, drain_per_pair=1)
                for sc in range(4 * j, 4 * j + 4):
                    for ec in range(4):
                        pending.append((sc, ec))
            else:
                attn_pass(j, 0, 0, NQ, drain_per_pair=1)
                attn_pass(j, 1, 0, NQ, drain_per_pair=1)
                for sc in range(4 * j, 4 * j + 4):
                    for ec in range(4):
                        pending.append((sc, ec))
                pools = [(psO, "O"), (psS, "S"), (psV, "V")]
                k = 0
                while pending:
                    pool, tag = pools[k % 3]
                    emit_outproj(*pending.pop(0), pool=pool, tag=tag, flush=True)
                    k += 1


_NC_CACHE = None


def _get_nc():
    global _NC_CACHE
    if _NC_CACHE is None:
        _NC_CACHE = build_nc()
    return _NC_CACHE


def _split8(a):
    hi = a.astype(E4)
    lo = (a - hi.astype(np.float32)).astype(E4)
    return hi, lo


def _pack_e(a):
    """[E, N] -> [128, EC, N] with e = c*128 + p."""
    n = a.shape[1]
    return np.ascontiguousarray(a.reshape(EC, 128, n).transpose(1, 0, 2))


def _prep_inputs(x, rotary_cos, rotary_sin, Wq, Wk, Wv, Wo):
    x = np.asarray(x, dtype=np.float32)[0]          # [S, E]
    cos = np.asarray(rotary_cos, dtype=np.float32)[0]  # [S, D]
    sin = np.asarray(rotary_sin, dtype=np.float32)[0]
    Wq = np.asarray(Wq, dtype=np.float32)
    Wk = np.asarray(Wk, dtype=np.float32)
    Wv = np.asarray(Wv, dtype=np.float32)
    Wo = np.asarray(Wo, dtype=np.float32)

    xT = np.ascontiguousarray(x.T)                   # [E, S]
    xh, xl = _split8(xT)
    xh = _pack_e(xh.astype(np.float32)).astype(E4)
    xl = _pack_e(xl.astype(np.float32)).astype(E4)

    cq = 1.0 / (SW * math.sqrt(math.sqrt(D)))
    cosP = np.ascontiguousarray(cos.T * cq).astype(np.float16)
    sinT = sin.T * cq
    sinP = np.concatenate([-sinT[:64], sinT[64:]], axis=0)
    sinP = np.ascontiguousarray(sinP).astype(np.float16)

    kk = np.arange(128)[:, None]
    qq = np.arange(128)[None, :]
    wedge = (kk <= qq).astype(np.float16)
    ones8 = np.ones((128, 16), dtype=np.float16)
    ones16 = np.full((1, 128), SA / SV, dtype=np.float16)
    biasm2 = np.full((128, 1), -2.0, dtype=np.float32)

    in_maps = []
    for c in range(NCORES):
        fs = slice(F2 * c, F2 * (c + 1))
        wq_h, wq_l = _split8(Wq[fs, :].T * SW)       # [E, F2]
        wk_h, wk_l = _split8(Wk[fs, :].T * SW)
        wv_h, wv_l = _split8(Wv[fs, :].T * SV)
        wo_s = Wo[:, fs].T * SO                       # [F2, E]
        wo_h, wo_l = _split8(wo_s)
        pk = lambda a: _pack_e(a.astype(np.float32)).astype(E4).reshape(128, EC * F2)
        pko = lambda a: np.ascontiguousarray(
            a.astype(np.float32).reshape(HPC, 128, E).transpose(1, 0, 2)
        ).astype(E4)
        in_maps.append({
            "xh": xh, "xl": xl,
            "wqh": pk(wq_h), "wql": pk(wq_l),
            "wkh": pk(wk_h), "wkl": pk(wk_l),
            "wvh": pk(wv_h), "wvl": pk(wv_l),
            "woh": pko(wo_h), "wol": pko(wo_l),
            "cosP": cosP, "sinP": sinP, "wedge": wedge,
            "ones8": ones8, "ones16": ones16, "biasm2": biasm2,
        })
    return in_maps


def kernel(x, rotary_cos, rotary_sin, Wq, Wk, Wv, Wo, **run_kwargs):
    nc = _get_nc()
    in_maps = _prep_inputs(x, rotary_cos, rotary_sin, Wq, Wk, Wv, Wo)
    res = run_bass_kernel_spmd(nc, in_maps, core_ids=list(range(NCORES)), **run_kwargs)
    acc = np.zeros((S, E), dtype=np.float64)
    for r in res.results:
        acc += r["out"].astype(np.float64)
    full = (acc / OUT_SCALE).astype(np.float32).reshape(1, S, E)
    if run_kwargs:
        return full, res
    return full
